# revision 58
# baseline (speedup 1.0000x reference)
"""Trainium2 Bass kernel: per-superpixel mean of CNN features + linear head.

reference computes:
    sums[s, f]  = segment_sum(features, superpixel)      # 1024 segments
    out[s, c]   = (sums[s] / max(count_s, 1)) @ w_node.T # [1024, 21]

Algebraic restructure: project each pixel's 256-dim feature to the 22-dim
padded class space FIRST, then segment-sum the projections:
    out[s, c] = segsum(feats @ w_aug.T)[s, c] / count_s
The segment reduction is a [pix,22].T @ onehot[pix,1024] matmul per
128-pixel tile, accumulated in PSUM across tiles.

v2 layout choices (vs v1):
  * features are transposed to [256 f, pix] bf16 on the host, so the
    projection reads fT blocks directly as the PE stationary operand —
    no per-tile PE transposes, fast (FWL) weight loads, and half the
    HBM traffic of fp32.
  * superpixel labels and the iota row are encoded as distinct bf16 BIT
    PATTERNS (0x4000+v) so the onehot is_equal compare runs all-bf16
    yet stays exact.

v3 range packing: the PE streams moving data at 1 col/cycle total, and
the seg matmul only used 22 of 128 output rows. So pack FOUR range-
masked copies of pq into the stationary,
    pq4[p, 22v + c] = pq[p, c] * 1[label_p >> 8 == v],
and stream a mod-256 onehot of only 256 columns:
    acc[22v + c, s'] += sum_p pq4[p, 22v+c] * 1[label_p % 256 == s']
which equals the segment sum for sp = 256v + s' exactly. This cuts both
the PE seg stream and the DVE compare 4x. Masks are host-precomputed
per tile; the mask multiply is one broadcast-AP DVE op. A fraction of
onehots is built on the otherwise-idle GPSIMD via local_scatter.

Sharding: 512*512 pixels split evenly across 8 cores (segment-sum is
permutation-invariant). Each core emits a [128, 256] partial holding
4 ranges x 22 class rows; the host adds the partials, divides by
counts (np.bincount) and transposes.
"""

import os as _os

import numpy as np
import ml_dtypes

import concourse.mybir as mybir
import concourse.tile as tile
from concourse import bacc
from concourse.bass_utils import run_bass_kernel_spmd

N_CORES = 8
P = 128
F = 256                      # feature dim
NUM_SP = 1024                # superpixel labels
C = 21                       # classes
CP = 22                      # classes padded even
NPIX = 512 * 512
PIX_PER_CORE = NPIX // N_CORES       # 32768
N_TILES = PIX_PER_CORE // P          # 256

CHUNK_PIX = int(_os.environ.get("KERNEL_CHUNK_PIX", "2048"))
N_CHUNKS = PIX_PER_CORE // CHUNK_PIX
TILES_PER_CHUNK = CHUNK_PIX // P

NR = 4                       # label ranges packed into the stationary
NSEG = NUM_SP // NR          # onehot width after range packing (256)

QB = int(_os.environ.get("KERNEL_QB", "4"))  # evac/mask batch width (tiles)

# onehot generation split: in every 8-tile group, the last GP_TAKE tiles'
# onehots are built by one batched gpsimd local_scatter, the rest by DVE
GP_PERIOD = 8
GP_TAKE = int(_os.environ.get("KERNEL_GP_TAKE", "6"))
GP_IDX = GP_TAKE + GP_TAKE % 2       # num_idxs must be even; pad with -1
N_GROUPS = N_TILES // GP_PERIOD

F32 = mybir.dt.float32
BF16 = mybir.dt.bfloat16
F8 = mybir.dt.float8e3

# features in fp8 e3m4 (scaled by FP8_SCALE, clipped to the e3m4 range)
# halve HBM traffic again vs bf16; w stays bf16 so the projection error
# is feature-quantization only (~1.2e-2 rel, gate is 2e-2)
USE_FP8 = bool(int(_os.environ.get("KERNEL_FP8", "1")))
FP8_SCALE = 3.0
FDT = F8 if USE_FP8 else BF16


def _build_nc():
    work_bufs = int(_os.environ.get("KERNEL_WORK_BUFS", "6"))
    psum_bufs = int(_os.environ.get("KERNEL_PSUM_BUFS", "3"))
    chunk_bufs = int(_os.environ.get("KERNEL_CHUNK_BUFS", "3"))
    split_first = bool(int(_os.environ.get("KERNEL_SPLIT_FIRST", "1")))
    nc = bacc.Bacc("TRN2", target_bir_lowering=False)

    feats = nc.dram_tensor(
        "feats", [N_CHUNKS, P, 2, CHUNK_PIX], FDT, kind="ExternalInput"
    )
    labels = nc.dram_tensor("labels", [P, N_TILES], F32, kind="ExternalInput")
    labels16 = nc.dram_tensor(
        "labels16", [P, N_GROUPS, GP_IDX], mybir.dt.int16, kind="ExternalInput"
    )
    iota = nc.dram_tensor("iota", [P, NSEG], BF16, kind="ExternalInput")
    mask4 = nc.dram_tensor(
        "mask4", [P, N_TILES // QB, QB, NR, 1], BF16, kind="ExternalInput"
    )
    w_aug = nc.dram_tensor("w_aug", [P, 2, CP], BF16, kind="ExternalInput")
    out = nc.dram_tensor("out", [P, NSEG], F32, kind="ExternalOutput")

    with tile.TileContext(nc) as tc:
        with (
            tc.tile_pool(name="const", bufs=1) as const_pool,
            tc.tile_pool(name="chunk", bufs=chunk_bufs) as chunk_pool,
            tc.tile_pool(name="work", bufs=work_bufs) as work_pool,
            tc.tile_pool(name="psum", bufs=psum_bufs, space="PSUM") as psum_pool,
            tc.tile_pool(name="accp", bufs=1, space="PSUM") as acc_pool,
        ):
            # head order: the first proj needs only w + the first feats
            # piece, so those two lead the sync queue; iota/labels (needed
            # by the first onehot, consumed a few tiles later) follow
            w_sb = const_pool.tile([P, 2, CP], BF16)
            nc.sync.dma_start(out=w_sb[:], in_=w_aug[:])
            feats0_sb = chunk_pool.tile([P, 2, CHUNK_PIX], FDT, tag="feats")
            nc.sync.dma_start(
                out=feats0_sb[:, :, 0:512], in_=feats[0][:, :, 0:512]
            )
            iota_sb = const_pool.tile([P, NSEG], BF16)
            nc.sync.dma_start(out=iota_sb[:], in_=iota[:])
            labels_sb = const_pool.tile([P, N_TILES], F32)
            nc.sync.dma_start(out=labels_sb[:], in_=labels[:])
            # bulky consts ride the Activation HWDGE queue (emitted in the
            # chunk-0 loop below, overlapping the first feature pieces)
            mask4_sb = const_pool.tile([P, N_TILES // QB, QB, NR, 1], BF16)
            if GP_TAKE:
                labels16_sb = const_pool.tile(
                    [P, N_GROUPS, GP_IDX], mybir.dt.int16
                )
                ones_sb = const_pool.tile([P, GP_IDX], BF16)
                nc.gpsimd.memset(ones_sb[:], 1.0)

            # persistent accumulator: acc[22v + c, s'] = partial sum for
            # superpixel 256v + s', class c
            acc = acc_pool.tile([P, NSEG], F32)

            state = {"proj_ps": None, "pq4": None, "ohgp": None}

            def emit_proj(tg, feats_sb, col):
                # proj[pix, c] = sum_f fT[f, pix] * w_aug[f, c]; tile quads
                # share one PSUM tile so evac and mask run once per 4 tiles
                if tg % QB == 0:
                    state["proj_ps"] = psum_pool.tile(
                        [P, QB, CP], F32, tag="projps", name="proj_ps"
                    )
                proj_ps = state["proj_ps"]
                j = tg % QB
                nc.tensor.matmul(
                    out=proj_ps[:, j, :],
                    lhsT=feats_sb[:, 0, col : col + P],
                    rhs=w_sb[:, 0, :],
                    start=True,
                    stop=False,
                    skip_group_check=True,
                )
                nc.tensor.matmul(
                    out=proj_ps[:, j, :],
                    lhsT=feats_sb[:, 1, col : col + P],
                    rhs=w_sb[:, 1, :],
                    start=False,
                    stop=True,
                    skip_group_check=True,
                )
                if j == QB - 1:
                    pq_sb = work_pool.tile([P, QB, 1, CP], BF16, tag="pqsb")
                    nc.scalar.activation(
                        out=pq_sb[:, :, 0, :],
                        in_=proj_ps[:],
                        func=mybir.ActivationFunctionType.Copy,
                    )
                    # pq4[p, t, 22v+c] = pq[p, t, c] * mask4[p, quad, t, v]
                    pq4_sb = work_pool.tile([P, QB, NR, CP], BF16, tag="pq4sb")
                    nc.vector.tensor_tensor(
                        out=pq4_sb[:],
                        in0=pq_sb[:].broadcast_to([P, QB, NR, CP]),
                        in1=mask4_sb[:, tg // QB, :, :, :].broadcast_to(
                            [P, QB, NR, CP]
                        ),
                        op=mybir.AluOpType.mult,
                    )
                    state["pq4"] = pq4_sb

            def emit_onehot(tg):
                # onehot[p, s'] = (iota[p, s'] == label[p] % 256); all-bf16
                # bit-pattern compare (exact). The last GP_TAKE tiles of each
                # 8-tile group come from one batched gpsimd local_scatter
                # (emitted a group early, see emit_gp_batch).
                k = tg % GP_PERIOD
                if GP_TAKE and k >= GP_PERIOD - GP_TAKE:
                    return state["ohgp"][:, k - (GP_PERIOD - GP_TAKE), :]
                onehot = work_pool.tile([P, NSEG], BF16, tag="onehot")
                nc.vector.tensor_scalar(
                    onehot[:],
                    iota_sb[:],
                    labels_sb[:, tg : tg + 1],
                    None,
                    mybir.AluOpType.is_equal,
                )
                return onehot[:]

            def emit_gp_batch(grp):
                # one local_scatter builds GP_TAKE onehots: index j's value
                # is label%256 + 256*j, scattered into a [P, GP_TAKE*256] buf
                ohgp = work_pool.tile([P, GP_TAKE, NSEG], BF16, tag="ohgp")
                nc.gpsimd.local_scatter(
                    out_ap=ohgp[:],
                    data_ap=ones_sb[:],
                    idxs_ap=labels16_sb[:, grp, :],
                    channels=P,
                    num_elems=GP_TAKE * NSEG,
                    num_idxs=GP_IDX,
                )
                state["ohgp"] = ohgp

            def emit_seg(tg, pq4_sb, onehot_ap):
                # acc[22v + c, s'] += pq4[p, 22v + c] * onehot[p, s']
                nc.tensor.matmul(
                    out=acc[0 : NR * CP, :],
                    lhsT=pq4_sb,
                    rhs=onehot_ap,
                    start=tg == 0,
                    stop=tg == N_TILES - 1,
                    tile_position=(0, 0),
                    skip_group_check=True,
                )

            # software pipelining: the PE queue is strict FIFO, so seg(t) at
            # the queue head waiting on evac(t) would block proj(t+1). Emit
            # seg(t) only after proj(t+skew) so evacs finish off-path.
            # skew must be >= QB so a tile's quad-batched pq4 exists.
            skew = max(QB, int(_os.environ.get("KERNEL_SKEW", "4")))
            oh_by_tg = {}
            pq4_by_tg = {}

            def flush_seg(s):
                emit_seg(s, pq4_by_tg.pop(s), oh_by_tg.pop(s))

            for c in range(N_CHUNKS):
                if c == 0:
                    # piece [0:512] was issued at the head; fetch the rest
                    feats_sb = feats0_sb
                    nc.sync.dma_start(
                        out=feats_sb[:, :, 512:CHUNK_PIX],
                        in_=feats[c][:, :, 512:CHUNK_PIX],
                    )
                    nc.scalar.dma_start(out=mask4_sb[:], in_=mask4[:])
                    if GP_TAKE:
                        nc.scalar.dma_start(
                            out=labels16_sb[:], in_=labels16[:]
                        )
                else:
                    feats_sb = chunk_pool.tile(
                        [P, 2, CHUNK_PIX], FDT, tag="feats"
                    )
                    for h in range(2):
                        nc.sync.dma_start(
                            out=feats_sb[:, h, :], in_=feats[c][:, h, :]
                        )

                for t in range(TILES_PER_CHUNK):
                    tg = c * TILES_PER_CHUNK + t
                    if GP_TAKE and tg % GP_PERIOD == 0:
                        emit_gp_batch(tg // GP_PERIOD)
                    emit_proj(tg, feats_sb, t * P)
                    oh_by_tg[tg] = emit_onehot(tg)
                    if tg % QB == QB - 1:
                        pq4_sb = state["pq4"]
                        for jj in range(QB):
                            pq4_by_tg[tg - QB + 1 + jj] = pq4_sb[:, jj, :, :]
                    if tg - skew >= 0 and tg - skew in pq4_by_tg:
                        flush_seg(tg - skew)
            for s in sorted(pq4_by_tg):
                flush_seg(s)

            out_sb = chunk_pool.tile([P, NSEG], F32, tag="outsb")
            nc.scalar.activation(
                out=out_sb[:], in_=acc[:], func=mybir.ActivationFunctionType.Copy
            )
            nc.sync.dma_start(out=out[:], in_=out_sb[:])

    nc.compile()
    return nc


def _install_ntff_hook():
    """Register the axon NTFF profiling hook when the image's antenv
    lacks axon_hooks (mirrors trn_agent_boot._ntff_profile_via_ctypes)."""
    import contextlib
    import ctypes
    import sys
    import types

    if "antenv.axon_hooks" in sys.modules:
        return
    lib = ctypes.CDLL("/opt/axon/libaxon_pjrt.so")
    if not hasattr(lib, "axon_start_nrt_profile"):
        return
    lib.axon_start_nrt_profile.argtypes = [
        ctypes.POINTER(ctypes.c_int64),
        ctypes.c_size_t,
    ]
    lib.axon_start_nrt_profile.restype = ctypes.c_int64
    lib.axon_stop_nrt_profile.argtypes = [ctypes.c_char_p]
    lib.axon_stop_nrt_profile.restype = ctypes.c_int64

    @contextlib.contextmanager
    def _hook(output_dir, device_ids):
        import jax

        jax.devices()
        if device_ids:
            ids = (ctypes.c_int64 * len(device_ids))(*device_ids)
            rc = lib.axon_start_nrt_profile(ids, len(device_ids))
        else:
            rc = lib.axon_start_nrt_profile(None, 0)
        if rc != 0:
            raise RuntimeError(f"axon_start_nrt_profile rc={rc}")
        try:
            yield
        finally:
            n = lib.axon_stop_nrt_profile(str(output_dir).encode())
            print(f"profile: {n} file(s) written to {output_dir}", file=sys.stderr)

    mod = types.ModuleType("antenv.axon_hooks")
    mod.get_axon_ntff_profile_hook = lambda: _hook
    mod.set_axon_ntff_profile_hook = lambda h: None
    sys.modules["antenv.axon_hooks"] = mod


_NC_CACHE = None


def _get_nc():
    global _NC_CACHE
    if _NC_CACHE is None:
        _NC_CACHE = _build_nc()
    return _NC_CACHE


def _encode_bf16_pattern(v):
    """Map small non-negative ints to distinct, exactly-comparable bf16
    bit patterns (0x4000 + v are all normal, distinct values)."""
    return (0x4000 + np.asarray(v, dtype=np.uint16)).view(ml_dtypes.bfloat16)


def kernel(features, superpixel, w_node):
    features = np.asarray(features, dtype=np.float32)
    superpixel = np.asarray(superpixel)
    w_node = np.asarray(w_node, dtype=np.float32)

    feats_flat = features.reshape(NPIX, F)
    sp_flat = superpixel.reshape(NPIX).astype(np.int64)

    # host-side layout: transposed features [256 f, NPIX pix]
    if USE_FP8:
        fq = np.clip(feats_flat * FP8_SCALE, -15.5, 15.5).astype(
            ml_dtypes.float8_e3m4
        )
    else:
        fq = feats_flat.astype(ml_dtypes.bfloat16)
    fT = np.ascontiguousarray(fq.T)
    sp_mod = sp_flat % NSEG
    sp_rng = sp_flat // NSEG
    enc = _encode_bf16_pattern(sp_mod)

    wa = np.zeros((F, CP), dtype=np.float32)
    wa[:, :C] = w_node.T
    # w_aug dram layout [P, 2, CP]: [f_lo, h, c] = w_aug[128h + f_lo, c]
    wa_bf = np.ascontiguousarray(
        wa.astype(ml_dtypes.bfloat16).reshape(2, P, CP).transpose(1, 0, 2)
    )
    iota = np.ascontiguousarray(
        np.broadcast_to(_encode_bf16_pattern(np.arange(NSEG))[None, :], (P, NSEG))
    )

    in_maps = []
    for core in range(N_CORES):
        lo = core * PIX_PER_CORE
        fc = fT[:, lo : lo + PIX_PER_CORE]
        # feats[c, p, h, j] = fT[128h + p, lo + c*CHUNK_PIX + j]
        f_dev = np.ascontiguousarray(
            fc.reshape(2, P, N_CHUNKS, CHUNK_PIX).transpose(2, 1, 0, 3)
        )
        # labels[p, tg] = enc(sp_mod[lo + 128*tg + p]); scalar port is fp32,
        # bf16->fp32 is exact so the pattern compare still matches
        lab_core = enc[lo : lo + PIX_PER_CORE].reshape(N_TILES, P).T
        lab = np.ascontiguousarray(lab_core.astype(np.float32))
        # grouped int16 indices for the batched gpsimd local_scatter: in
        # group g, slot j covers tile 8g + (8 - GP_TAKE) + j, scattering
        # label%256 + 256*j into a [P, GP_TAKE*256] buffer
        mod_t = sp_mod[lo : lo + PIX_PER_CORE].reshape(N_TILES, P).T
        lab16 = np.full((P, N_GROUPS, GP_IDX), -1, dtype=np.int16)
        for j in range(GP_TAKE):
            tiles = np.arange(N_GROUPS) * GP_PERIOD + (GP_PERIOD - GP_TAKE) + j
            lab16[:, :, j] = mod_t[:, tiles] + NSEG * j
        # mask4[p, pair, t, v] = 1 if pixel's label is in range v (label>>8)
        rng = sp_rng[lo : lo + PIX_PER_CORE].reshape(N_TILES, P).T
        m4 = (rng[:, :, None] == np.arange(NR)[None, None, :]).astype(
            ml_dtypes.bfloat16
        )
        m4 = m4.reshape(P, N_TILES // QB, QB, NR)[..., None]
        in_maps.append(
            {
                "feats": f_dev,
                "labels": lab,
                "labels16": lab16,
                "iota": iota,
                "mask4": np.ascontiguousarray(m4),
                "w_aug": wa_bf,
            }
        )

    trace = bool(int(_os.environ.get("KERNEL_TRACE", "0")))
    repeat = int(_os.environ.get("KERNEL_REPEAT", "1"))
    kwargs = {}
    if trace:
        _install_ntff_hook()
        import concourse.bass_utils as _bu

        _bu.upload_artifacts = lambda tmpdir: tmpdir
    base_dir = _os.environ.get("KERNEL_TRACE_DIR") or None
    for rep in range(repeat):
        if trace and base_dir:
            kwargs["tmpdir"] = _os.path.join(base_dir, f"rep{rep}")
            _os.makedirs(kwargs["tmpdir"], exist_ok=True)
        res = run_bass_kernel_spmd(
            _get_nc(), in_maps, core_ids=list(range(N_CORES)), trace=trace, **kwargs
        )
        if trace:
            print(f"HW exec time: {res.exec_time_ns} ns")
            print(f"profile_json: {res.profile_json}")

    total = np.zeros((C, NUM_SP), dtype=np.float64)
    for r in res.results:
        o = np.asarray(r["out"], dtype=np.float64)
        for v in range(NR):
            total[:, NSEG * v : NSEG * (v + 1)] += o[CP * v : CP * v + C]
    counts = np.bincount(sp_flat, minlength=NUM_SP).astype(np.float64)
    if USE_FP8:
        total /= FP8_SCALE
    node_potentials = total / np.clip(counts, 1.0, None)
    return np.ascontiguousarray(node_potentials.T).astype(np.float32)


# revision 59
# speedup vs baseline: 1.0660x; 1.0660x over previous
"""Trainium2 Bass kernel: per-superpixel mean of CNN features + linear head.

reference computes:
    sums[s, f]  = segment_sum(features, superpixel)      # 1024 segments
    out[s, c]   = (sums[s] / max(count_s, 1)) @ w_node.T # [1024, 21]

Algebraic restructure: project each pixel's 256-dim feature to the 22-dim
padded class space FIRST, then segment-sum the projections:
    out[s, c] = segsum(feats @ w_aug.T)[s, c] / count_s
The segment reduction is a [pix,22].T @ onehot[pix,1024] matmul per
128-pixel tile, accumulated in PSUM across tiles.

v2 layout choices (vs v1):
  * features are transposed to [256 f, pix] bf16 on the host, so the
    projection reads fT blocks directly as the PE stationary operand —
    no per-tile PE transposes, fast (FWL) weight loads, and half the
    HBM traffic of fp32.
  * superpixel labels and the iota row are encoded as distinct bf16 BIT
    PATTERNS (0x4000+v) so the onehot is_equal compare runs all-bf16
    yet stays exact.

v3 range packing: the PE streams moving data at 1 col/cycle total, and
the seg matmul only used 22 of 128 output rows. So pack FOUR range-
masked copies of pq into the stationary,
    pq4[p, 22v + c] = pq[p, c] * 1[label_p >> 8 == v],
and stream a mod-256 onehot of only 256 columns:
    acc[22v + c, s'] += sum_p pq4[p, 22v+c] * 1[label_p % 256 == s']
which equals the segment sum for sp = 256v + s' exactly. This cuts both
the PE seg stream and the DVE compare 4x. Masks are host-precomputed
per tile; the mask multiply is one broadcast-AP DVE op (quad-batched
with the PSUM evacuation: one scalar evac + one DVE mult per 4 tiles).
6 of every 8 onehots are built on the otherwise-idle GPSIMD via one
batched local_scatter (indices offset by 256*j into a shared buffer).

v4 fp8: features ship as fp8 e3m4 (x3, clipped to +-15.5), halving HBM
traffic again; w stays bf16 (mixed-dtype matmul), so the error is
feature quantization only — measured 1.19e-2 rel vs the 2e-2 gate,
deterministic for the fixed-seed inputs.

Sharding: 512*512 pixels split evenly across 8 cores (segment-sum is
permutation-invariant). Each core emits a [128, 256] partial holding
4 ranges x 22 class rows; the host adds the partials, divides by
counts (np.bincount), rescales, and transposes.
"""

import os as _os

import numpy as np
import ml_dtypes

import concourse.mybir as mybir
import concourse.tile as tile
from concourse import bacc
from concourse.bass_utils import run_bass_kernel_spmd

N_CORES = 8
P = 128
F = 256                      # feature dim
NUM_SP = 1024                # superpixel labels
C = 21                       # classes
CP = 22                      # classes padded even
NPIX = 512 * 512
PIX_PER_CORE = NPIX // N_CORES       # 32768
N_TILES = PIX_PER_CORE // P          # 256

CHUNK_PIX = int(_os.environ.get("KERNEL_CHUNK_PIX", "2048"))
N_CHUNKS = PIX_PER_CORE // CHUNK_PIX
TILES_PER_CHUNK = CHUNK_PIX // P

NR = 4                       # label ranges packed into the stationary
NSEG = NUM_SP // NR          # onehot width after range packing (256)

QB = int(_os.environ.get("KERNEL_QB", "4"))  # evac/mask batch width (tiles)

# onehot generation split: in every 8-tile group, the last GP_TAKE tiles'
# onehots are built by one batched gpsimd local_scatter, the rest by DVE
GP_PERIOD = 8
GP_TAKE = int(_os.environ.get("KERNEL_GP_TAKE", "6"))
GP_IDX = GP_TAKE + GP_TAKE % 2       # num_idxs must be even; pad with -1
N_GROUPS = N_TILES // GP_PERIOD

F32 = mybir.dt.float32
BF16 = mybir.dt.bfloat16
F8 = mybir.dt.float8e3

# features in fp8 e3m4 (scaled by FP8_SCALE, clipped to the e3m4 range)
# halve HBM traffic again vs bf16; w stays bf16 so the projection error
# is feature-quantization only (~1.2e-2 rel, gate is 2e-2)
USE_FP8 = bool(int(_os.environ.get("KERNEL_FP8", "1")))
FP8_SCALE = 3.0
FDT = F8 if USE_FP8 else BF16


def _build_nc():
    work_bufs = int(_os.environ.get("KERNEL_WORK_BUFS", "6"))
    psum_bufs = int(_os.environ.get("KERNEL_PSUM_BUFS", "3"))
    chunk_bufs = int(_os.environ.get("KERNEL_CHUNK_BUFS", "3"))
    split_first = bool(int(_os.environ.get("KERNEL_SPLIT_FIRST", "1")))
    nc = bacc.Bacc("TRN2", target_bir_lowering=False)

    feats = nc.dram_tensor(
        "feats", [N_CHUNKS, P, 2, CHUNK_PIX], FDT, kind="ExternalInput"
    )
    labels = nc.dram_tensor("labels", [P, N_TILES], F32, kind="ExternalInput")
    labels16 = nc.dram_tensor(
        "labels16", [P, N_GROUPS, GP_IDX], mybir.dt.int16, kind="ExternalInput"
    )
    iota = nc.dram_tensor("iota", [P, NSEG], BF16, kind="ExternalInput")
    mask4 = nc.dram_tensor(
        "mask4", [P, N_TILES // QB, QB, NR, 1], BF16, kind="ExternalInput"
    )
    w_aug = nc.dram_tensor("w_aug", [P, 2, CP], BF16, kind="ExternalInput")
    out = nc.dram_tensor("out", [P, NSEG], F32, kind="ExternalOutput")

    with tile.TileContext(nc) as tc:
        with (
            tc.tile_pool(name="const", bufs=1) as const_pool,
            tc.tile_pool(name="chunk", bufs=chunk_bufs) as chunk_pool,
            tc.tile_pool(name="work", bufs=work_bufs) as work_pool,
            tc.tile_pool(name="psum", bufs=psum_bufs, space="PSUM") as psum_pool,
            tc.tile_pool(name="accp", bufs=1, space="PSUM") as acc_pool,
        ):
            # head order: the first proj needs only w + the first feats
            # piece, so those two lead the sync queue; iota/labels (needed
            # by the first onehot, consumed a few tiles later) follow
            w_sb = const_pool.tile([P, 2, CP], BF16)
            nc.sync.dma_start(out=w_sb[:], in_=w_aug[:])
            feats0_sb = chunk_pool.tile([P, 2, CHUNK_PIX], FDT, tag="feats")
            nc.sync.dma_start(
                out=feats0_sb[:, :, 0:512], in_=feats[0][:, :, 0:512]
            )
            iota_sb = const_pool.tile([P, NSEG], BF16)
            nc.sync.dma_start(out=iota_sb[:], in_=iota[:])
            labels_sb = const_pool.tile([P, N_TILES], F32)
            nc.sync.dma_start(out=labels_sb[:], in_=labels[:])
            # bulky consts ride the Activation HWDGE queue (emitted in the
            # chunk-0 loop below, overlapping the first feature pieces)
            mask4_sb = const_pool.tile([P, N_TILES // QB, QB, NR, 1], BF16)
            if GP_TAKE:
                labels16_sb = const_pool.tile(
                    [P, N_GROUPS, GP_IDX], mybir.dt.int16
                )
                ones_sb = const_pool.tile([P, GP_IDX], BF16)
                nc.gpsimd.memset(ones_sb[:], 1.0)

            # persistent accumulator: acc[22v + c, s'] = partial sum for
            # superpixel 256v + s', class c
            acc = acc_pool.tile([P, NSEG], F32)

            state = {"proj_ps": None, "pq4": None, "ohgp": None}

            def emit_proj(tg, feats_sb, col):
                # proj[pix, c] = sum_f fT[f, pix] * w_aug[f, c]; tile quads
                # share one PSUM tile so evac and mask run once per 4 tiles
                if tg % QB == 0:
                    state["proj_ps"] = psum_pool.tile(
                        [P, QB, CP], F32, tag="projps", name="proj_ps"
                    )
                proj_ps = state["proj_ps"]
                j = tg % QB
                nc.tensor.matmul(
                    out=proj_ps[:, j, :],
                    lhsT=feats_sb[:, 0, col : col + P],
                    rhs=w_sb[:, 0, :],
                    start=True,
                    stop=False,
                    skip_group_check=True,
                )
                nc.tensor.matmul(
                    out=proj_ps[:, j, :],
                    lhsT=feats_sb[:, 1, col : col + P],
                    rhs=w_sb[:, 1, :],
                    start=False,
                    stop=True,
                    skip_group_check=True,
                )
                if j == QB - 1:
                    pq_sb = work_pool.tile([P, QB, 1, CP], BF16, tag="pqsb")
                    nc.scalar.activation(
                        out=pq_sb[:, :, 0, :],
                        in_=proj_ps[:],
                        func=mybir.ActivationFunctionType.Copy,
                    )
                    # pq4[p, t, 22v+c] = pq[p, t, c] * mask4[p, quad, t, v]
                    pq4_sb = work_pool.tile([P, QB, NR, CP], BF16, tag="pq4sb")
                    nc.vector.tensor_tensor(
                        out=pq4_sb[:],
                        in0=pq_sb[:].broadcast_to([P, QB, NR, CP]),
                        in1=mask4_sb[:, tg // QB, :, :, :].broadcast_to(
                            [P, QB, NR, CP]
                        ),
                        op=mybir.AluOpType.mult,
                    )
                    state["pq4"] = pq4_sb

            def emit_onehot(tg):
                # onehot[p, s'] = (iota[p, s'] == label[p] % 256); all-bf16
                # bit-pattern compare (exact). The last GP_TAKE tiles of each
                # 8-tile group come from one batched gpsimd local_scatter
                # (emitted a group early, see emit_gp_batch).
                k = tg % GP_PERIOD
                if GP_TAKE and k >= GP_PERIOD - GP_TAKE:
                    return state["ohgp"][:, k - (GP_PERIOD - GP_TAKE), :]
                onehot = work_pool.tile([P, NSEG], BF16, tag="onehot")
                nc.vector.tensor_scalar(
                    onehot[:],
                    iota_sb[:],
                    labels_sb[:, tg : tg + 1],
                    None,
                    mybir.AluOpType.is_equal,
                )
                return onehot[:]

            def emit_gp_batch(grp):
                # one local_scatter builds GP_TAKE onehots: index j's value
                # is label%256 + 256*j, scattered into a [P, GP_TAKE*256] buf
                ohgp = work_pool.tile([P, GP_TAKE, NSEG], BF16, tag="ohgp")
                nc.gpsimd.local_scatter(
                    out_ap=ohgp[:],
                    data_ap=ones_sb[:],
                    idxs_ap=labels16_sb[:, grp, :],
                    channels=P,
                    num_elems=GP_TAKE * NSEG,
                    num_idxs=GP_IDX,
                )
                state["ohgp"] = ohgp

            def emit_seg(tg, pq4_sb, onehot_ap):
                # acc[22v + c, s'] += pq4[p, 22v + c] * onehot[p, s']
                nc.tensor.matmul(
                    out=acc[0 : NR * CP, :],
                    lhsT=pq4_sb,
                    rhs=onehot_ap,
                    start=tg == 0,
                    stop=tg == N_TILES - 1,
                    tile_position=(0, 0),
                    skip_group_check=True,
                )

            # software pipelining: the PE queue is strict FIFO, so seg(t) at
            # the queue head waiting on evac(t) would block proj(t+1). Emit
            # seg(t) only after proj(t+skew) so evacs finish off-path.
            # skew must be >= QB so a tile's quad-batched pq4 exists.
            skew = max(QB, int(_os.environ.get("KERNEL_SKEW", "4")))
            oh_by_tg = {}
            pq4_by_tg = {}

            def flush_seg(s):
                emit_seg(s, pq4_by_tg.pop(s), oh_by_tg.pop(s))

            for c in range(N_CHUNKS):
                if c == 0:
                    # piece [0:512] was issued at the head; fetch the rest
                    feats_sb = feats0_sb
                    nc.sync.dma_start(
                        out=feats_sb[:, :, 512:CHUNK_PIX],
                        in_=feats[c][:, :, 512:CHUNK_PIX],
                    )
                    nc.scalar.dma_start(out=mask4_sb[:], in_=mask4[:])
                    if GP_TAKE:
                        nc.scalar.dma_start(
                            out=labels16_sb[:], in_=labels16[:]
                        )
                else:
                    feats_sb = chunk_pool.tile(
                        [P, 2, CHUNK_PIX], FDT, tag="feats"
                    )
                    for h in range(2):
                        nc.sync.dma_start(
                            out=feats_sb[:, h, :], in_=feats[c][:, h, :]
                        )

                for t in range(TILES_PER_CHUNK):
                    tg = c * TILES_PER_CHUNK + t
                    if GP_TAKE and tg % GP_PERIOD == 0:
                        emit_gp_batch(tg // GP_PERIOD)
                    emit_proj(tg, feats_sb, t * P)
                    oh_by_tg[tg] = emit_onehot(tg)
                    if tg % QB == QB - 1:
                        pq4_sb = state["pq4"]
                        for jj in range(QB):
                            pq4_by_tg[tg - QB + 1 + jj] = pq4_sb[:, jj, :, :]
                    if tg - skew >= 0 and tg - skew in pq4_by_tg:
                        flush_seg(tg - skew)
            for s in sorted(pq4_by_tg):
                flush_seg(s)

            out_sb = chunk_pool.tile([P, NSEG], F32, tag="outsb")
            nc.scalar.activation(
                out=out_sb[:], in_=acc[:], func=mybir.ActivationFunctionType.Copy
            )
            nc.sync.dma_start(out=out[:], in_=out_sb[:])

    nc.compile()
    return nc


def _install_ntff_hook():
    """Register the axon NTFF profiling hook when the image's antenv
    lacks axon_hooks (mirrors trn_agent_boot._ntff_profile_via_ctypes)."""
    import contextlib
    import ctypes
    import sys
    import types

    if "antenv.axon_hooks" in sys.modules:
        return
    lib = ctypes.CDLL("/opt/axon/libaxon_pjrt.so")
    if not hasattr(lib, "axon_start_nrt_profile"):
        return
    lib.axon_start_nrt_profile.argtypes = [
        ctypes.POINTER(ctypes.c_int64),
        ctypes.c_size_t,
    ]
    lib.axon_start_nrt_profile.restype = ctypes.c_int64
    lib.axon_stop_nrt_profile.argtypes = [ctypes.c_char_p]
    lib.axon_stop_nrt_profile.restype = ctypes.c_int64

    @contextlib.contextmanager
    def _hook(output_dir, device_ids):
        import jax

        jax.devices()
        if device_ids:
            ids = (ctypes.c_int64 * len(device_ids))(*device_ids)
            rc = lib.axon_start_nrt_profile(ids, len(device_ids))
        else:
            rc = lib.axon_start_nrt_profile(None, 0)
        if rc != 0:
            raise RuntimeError(f"axon_start_nrt_profile rc={rc}")
        try:
            yield
        finally:
            n = lib.axon_stop_nrt_profile(str(output_dir).encode())
            print(f"profile: {n} file(s) written to {output_dir}", file=sys.stderr)

    mod = types.ModuleType("antenv.axon_hooks")
    mod.get_axon_ntff_profile_hook = lambda: _hook
    mod.set_axon_ntff_profile_hook = lambda h: None
    sys.modules["antenv.axon_hooks"] = mod


_NC_CACHE = None


def _get_nc():
    global _NC_CACHE
    if _NC_CACHE is None:
        _NC_CACHE = _build_nc()
    return _NC_CACHE


def _encode_bf16_pattern(v):
    """Map small non-negative ints to distinct, exactly-comparable bf16
    bit patterns (0x4000 + v are all normal, distinct values)."""
    return (0x4000 + np.asarray(v, dtype=np.uint16)).view(ml_dtypes.bfloat16)


def kernel(features, superpixel, w_node):
    features = np.asarray(features, dtype=np.float32)
    superpixel = np.asarray(superpixel)
    w_node = np.asarray(w_node, dtype=np.float32)

    feats_flat = features.reshape(NPIX, F)
    sp_flat = superpixel.reshape(NPIX).astype(np.int64)

    # host-side layout: transposed features [256 f, NPIX pix]
    if USE_FP8:
        fq = np.clip(feats_flat * FP8_SCALE, -15.5, 15.5).astype(
            ml_dtypes.float8_e3m4
        )
    else:
        fq = feats_flat.astype(ml_dtypes.bfloat16)
    fT = np.ascontiguousarray(fq.T)
    sp_mod = sp_flat % NSEG
    sp_rng = sp_flat // NSEG
    enc = _encode_bf16_pattern(sp_mod)

    wa = np.zeros((F, CP), dtype=np.float32)
    wa[:, :C] = w_node.T
    # w_aug dram layout [P, 2, CP]: [f_lo, h, c] = w_aug[128h + f_lo, c]
    wa_bf = np.ascontiguousarray(
        wa.astype(ml_dtypes.bfloat16).reshape(2, P, CP).transpose(1, 0, 2)
    )
    iota = np.ascontiguousarray(
        np.broadcast_to(_encode_bf16_pattern(np.arange(NSEG))[None, :], (P, NSEG))
    )

    in_maps = []
    for core in range(N_CORES):
        lo = core * PIX_PER_CORE
        fc = fT[:, lo : lo + PIX_PER_CORE]
        # feats[c, p, h, j] = fT[128h + p, lo + c*CHUNK_PIX + j]
        f_dev = np.ascontiguousarray(
            fc.reshape(2, P, N_CHUNKS, CHUNK_PIX).transpose(2, 1, 0, 3)
        )
        # labels[p, tg] = enc(sp_mod[lo + 128*tg + p]); scalar port is fp32,
        # bf16->fp32 is exact so the pattern compare still matches
        lab_core = enc[lo : lo + PIX_PER_CORE].reshape(N_TILES, P).T
        lab = np.ascontiguousarray(lab_core.astype(np.float32))
        # grouped int16 indices for the batched gpsimd local_scatter: in
        # group g, slot j covers tile 8g + (8 - GP_TAKE) + j, scattering
        # label%256 + 256*j into a [P, GP_TAKE*256] buffer
        mod_t = sp_mod[lo : lo + PIX_PER_CORE].reshape(N_TILES, P).T
        lab16 = np.full((P, N_GROUPS, GP_IDX), -1, dtype=np.int16)
        for j in range(GP_TAKE):
            tiles = np.arange(N_GROUPS) * GP_PERIOD + (GP_PERIOD - GP_TAKE) + j
            lab16[:, :, j] = mod_t[:, tiles] + NSEG * j
        # mask4[p, pair, t, v] = 1 if pixel's label is in range v (label>>8)
        rng = sp_rng[lo : lo + PIX_PER_CORE].reshape(N_TILES, P).T
        m4 = (rng[:, :, None] == np.arange(NR)[None, None, :]).astype(
            ml_dtypes.bfloat16
        )
        m4 = m4.reshape(P, N_TILES // QB, QB, NR)[..., None]
        in_maps.append(
            {
                "feats": f_dev,
                "labels": lab,
                "labels16": lab16,
                "iota": iota,
                "mask4": np.ascontiguousarray(m4),
                "w_aug": wa_bf,
            }
        )

    trace = bool(int(_os.environ.get("KERNEL_TRACE", "0")))
    repeat = int(_os.environ.get("KERNEL_REPEAT", "1"))
    kwargs = {}
    if trace:
        _install_ntff_hook()
        import concourse.bass_utils as _bu

        _bu.upload_artifacts = lambda tmpdir: tmpdir
    base_dir = _os.environ.get("KERNEL_TRACE_DIR") or None
    for rep in range(repeat):
        if trace and base_dir:
            kwargs["tmpdir"] = _os.path.join(base_dir, f"rep{rep}")
            _os.makedirs(kwargs["tmpdir"], exist_ok=True)
        res = run_bass_kernel_spmd(
            _get_nc(), in_maps, core_ids=list(range(N_CORES)), trace=trace, **kwargs
        )
        if trace:
            print(f"HW exec time: {res.exec_time_ns} ns")
            print(f"profile_json: {res.profile_json}")

    total = np.zeros((C, NUM_SP), dtype=np.float64)
    for r in res.results:
        o = np.asarray(r["out"], dtype=np.float64)
        for v in range(NR):
            total[:, NSEG * v : NSEG * (v + 1)] += o[CP * v : CP * v + C]
    counts = np.bincount(sp_flat, minlength=NUM_SP).astype(np.float64)
    if USE_FP8:
        total /= FP8_SCALE
    node_potentials = total / np.clip(counts, 1.0, None)
    return np.ascontiguousarray(node_potentials.T).astype(np.float32)
